# revision 1
# baseline (speedup 1.0000x reference)
# Trainium2 Bass kernel for nn_EncoderRNN (bidirectional LSTM + attention + classifier).
#
# Sharding: data-parallel over batch (B=128 -> 16 per core, 8 cores), both LSTM
# directions computed per core, weights replicated. The sequential time scan
# stays local per shard.
#
# Self-contained: hardcodes shapes; takes full inputs, returns full output.
import numpy as np
import ml_dtypes

B, L, E, H, C = 128, 512, 512, 512, 16
NCORES = 8
BS = B // NCORES          # batch per core
W = 4                     # timesteps per x-precompute window
NW = L // W               # windows
UNROLL = 16               # For_i unroll factor
KC_H = H // 128           # h-part contraction chunks (4)
KC_E = E // 128           # x-part contraction chunks (4)
NMT = 4 * H // 128        # gate M-tiles (16); mt = gg*4 + hc
TOKCH = 16                # attention token chunks (512 tokens each)
TOKL = L // TOKCH         # l-range per token chunk (32)

_cache = {}
DEBUG = False


def _build_nc(rec_reps=1, attn_reps=1):
    import concourse.bacc as bacc
    import concourse.mybir as mybir
    import concourse.tile as tile
    from concourse.bass import ds
    import contextlib

    f32 = mybir.dt.float32
    bf16 = mybir.dt.bfloat16
    AF = mybir.ActivationFunctionType
    ALU = mybir.AluOpType
    AX = mybir.AxisListType

    nc = bacc.Bacc("TRN2", target_bir_lowering=False, debug=False,
                   num_devices=NCORES)

    # ---- I/O ----
    # x pre-transposed on host into window-major layout [NW, E, W, BS]
    xT = nc.dram_tensor("xT", [NW, E, W, BS], bf16, kind="ExternalInput").ap()
    xTr = nc.dram_tensor("xTr", [NW, E, W, BS], bf16, kind="ExternalInput").ap()
    wf = nc.dram_tensor("wf", [E + H, 4 * H], bf16, kind="ExternalInput").ap()
    wb = nc.dram_tensor("wb", [E + H, 4 * H], bf16, kind="ExternalInput").ap()
    bias_blk = nc.dram_tensor("bias_blk", [2, 16, 128], bf16,
                              kind="ExternalInput").ap()
    ind = nc.dram_tensor("ind", [16, 1024], bf16, kind="ExternalInput").ap()
    aw = nc.dram_tensor("aw", [2 * H, 2 * H], bf16, kind="ExternalInput").ap()
    ab_t = nc.dram_tensor("ab_t", [128, 2 * H // 128], f32, kind="ExternalInput").ap()
    av_t = nc.dram_tensor("av_t", [128, 2 * H // 128], bf16, kind="ExternalInput").ap()
    cw = nc.dram_tensor("cw", [2 * H, C], f32, kind="ExternalInput").ap()
    cb_rep = nc.dram_tensor("cb_rep", [BS, C], f32, kind="ExternalInput").ap()
    maskadd = nc.dram_tensor("maskadd", [BS, L], f32, kind="ExternalInput").ap()
    out = nc.dram_tensor("out", [BS, C], f32, kind="ExternalOutput").ap()

    wdr = [wf, wb]
    xv = [xT, xTr]

    with tile.TileContext(nc) as tc:
        with contextlib.ExitStack() as ctx:
            dramp = ctx.enter_context(tc.tile_pool(name="dram", bufs=1, space="DRAM"))
            # hid[ch][p, l, b]; ch = dir*4 + hc (h2 = ch*128 + p)
            if DEBUG:
                hid = nc.dram_tensor("hid_dbg", [8, 128, L, BS], bf16,
                                     kind="ExternalOutput").ap()
                alpha_d = nc.dram_tensor("alpha_dbg", [L, BS], f32,
                                         kind="ExternalOutput").ap()
                attw_d = nc.dram_tensor("attw_dbg", [L, BS], bf16,
                                        kind="ExternalOutput").ap()
            else:
                hid = dramp.tile([8, 128, L, BS], bf16)
                alpha_d = dramp.tile([L, BS], f32)
                attw_d = dramp.tile([L, BS], bf16)

            # ================= Phase B: bidirectional LSTM =================
            with contextlib.ExitStack() as rctx:
                wpool = rctx.enter_context(tc.tile_pool(name="wp", bufs=1))
                xpool = rctx.enter_context(tc.tile_pool(name="xp", bufs=2))
                spool = rctx.enter_context(tc.tile_pool(name="sp", bufs=3))
                ppool = rctx.enter_context(
                    tc.tile_pool(name="pp", bufs=2, space="PSUM"))

                # weights: [128, 8 kc, 2048] per dir (kc 0-3: x, 4-7: h)
                w_sb = []
                for d in range(2):
                    t = wpool.tile([128, 8, 4 * H], bf16, tag=f"w{d}")
                    for kc in range(8):
                        nc.sync.dma_start(
                            out=t[:, kc, :],
                            in_=wdr[d][kc * 128:(kc + 1) * 128, :])
                    w_sb.append(t)
                bb_sb = []
                for d in range(2):
                    t = wpool.tile([16, 128], bf16, tag=f"bb{d}")
                    nc.sync.dma_start(out=t, in_=bias_blk[d])
                    bb_sb.append(t)
                ind_sb = wpool.tile([16, 1024], bf16, tag="ind")
                nc.sync.dma_start(out=ind_sb, in_=ind)

                # recurrent state
                h_bf = []
                c_st = []
                for d in range(2):
                    hbt = wpool.tile([128, KC_H, BS], bf16, tag=f"h{d}")
                    nc.vector.memset(hbt, 0.0)
                    h_bf.append(hbt)
                    cst = wpool.tile([128, KC_H, BS], f32, tag=f"c{d}")
                    nc.vector.memset(cst, 0.0)
                    c_st.append(cst)

                stg_state = {}

                def window(wi, k):
                    psums = []
                    if k % 2 == 0:
                        stg_state["stg"] = [
                            spool.tile([128, KC_H, 2 * W, BS], bf16,
                                       name=f"stg{d}", tag=f"stg{d}")
                            for d in range(2)]
                    stgs = stg_state["stg"]
                    for d in range(2):
                        x_sb = xpool.tile([128, KC_E, W, BS], bf16, tag=f"x{d}")
                        for ec in range(KC_E):
                            nc.sync.dma_start(
                                out=x_sb[:, ec, :, :],
                                in_=xv[d][ds(wi, 1),
                                          ec * 128:(ec + 1) * 128,
                                          :, :].squeeze(0))
                        # psum [128, hc, gg, t, b]
                        ps = ppool.tile([128, KC_H, 4, W, BS], f32, tag=f"ps{d}")
                        psums.append(ps)
                        # bank openers: write whole bank (bias values) with
                        # start=True so everything after purely accumulates
                        psflat = ps.rearrange("p hc gg t b -> p (hc gg t b)")
                        for bank in range(2):
                            nc.tensor.matmul(
                                psflat[:, bank * 512:(bank + 1) * 512],
                                bb_sb[d][:, :],
                                ind_sb[:, bank * 512:(bank + 1) * 512],
                                start=True, stop=False, skip_group_check=True)
                        xflat = x_sb.rearrange("p e t b -> p e (t b)")
                        for ec in range(KC_E):
                            for mt in range(NMT):
                                gg, hc = mt // 4, mt % 4
                                nc.tensor.matmul(
                                    ps[:, hc, gg, :, :],
                                    w_sb[d][:, ec, mt * 128:(mt + 1) * 128],
                                    xflat[:, ec, :],
                                    start=False, stop=False,
                                    skip_group_check=True)

                    for ti in range(W):
                        for d in range(2):
                            ps = psums[d]
                            for kc in range(KC_H):
                                for mt in range(NMT):
                                    gg, hc = mt // 4, mt % 4
                                    nc.tensor.matmul(
                                        ps[:, hc, gg, ti, :],
                                        w_sb[d][:, 4 + kc,
                                                mt * 128:(mt + 1) * 128],
                                        h_bf[d][:, kc, :],
                                        start=False, stop=False,
                                        skip_group_check=True)
                            fio = spool.tile([128, KC_H, 3, BS], f32,
                                             tag=f"fio{d}")
                            nc.scalar.activation(fio, ps[:, :, 0:3, ti, :],
                                                 AF.Sigmoid)
                            g_s = spool.tile([128, KC_H, BS], f32, tag=f"g{d}")
                            nc.scalar.activation(g_s, ps[:, :, 3, ti, :],
                                                 AF.Tanh)
                            ig = spool.tile([128, KC_H, BS], f32, tag=f"ig{d}")
                            nc.vector.tensor_mul(ig, fio[:, :, 1, :], g_s)
                            fc = spool.tile([128, KC_H, BS], f32, tag=f"fc{d}")
                            nc.vector.tensor_mul(fc, fio[:, :, 0, :], c_st[d])
                            nc.vector.tensor_add(c_st[d], ig, fc)
                            tc_s = spool.tile([128, KC_H, BS], f32,
                                              tag=f"tc{d}")
                            nc.scalar.activation(tc_s, c_st[d], AF.Tanh)
                            nc.vector.tensor_mul(h_bf[d], fio[:, :, 2, :],
                                                 tc_s)
                            # stage h for the pair-batched hid write; bwd
                            # occupies reversed slots so dst times ascend
                            sl = (k % 2) * W + ti
                            slot = sl if d == 0 else 2 * W - 1 - sl
                            nc.vector.tensor_copy(stgs[d][:, :, slot, :],
                                                  h_bf[d])
                    if k % 2 == 1:
                        wbase = wi - 1  # symbolic start of the pair
                        for d in range(2):
                            td0 = (wbase * W if d == 0
                                   else L - 2 * W - wbase * W)
                            for hc in range(KC_H):
                                nc.sync.dma_start(
                                    out=hid[d * 4 + hc, :, ds(td0, 2 * W), :],
                                    in_=stgs[d][:, hc, :, :])

                def unroll_body(iv0, unroll):
                    assert unroll % 2 == 0, unroll
                    for k in range(unroll):
                        window(iv0 + k, k)

                if rec_reps == 1:
                    tc.For_i_unrolled_general(
                        0, NW, 1, unrollable_body=unroll_body,
                        max_unroll=UNROLL)
                else:
                    with tc.For_i(0, rec_reps) as _r:
                        tc.For_i_unrolled_general(
                            0, NW, 1, unrollable_body=unroll_body,
                            max_unroll=UNROLL)

            # ================= Phase C: attention + classifier =============
            actx_loop = tc.For_i(0, attn_reps) if attn_reps > 1 else None
            if actx_loop is not None:
                actx_loop.__enter__()
            with contextlib.ExitStack() as actx:
                cpool = actx.enter_context(tc.tile_pool(name="cp", bufs=1))
                hpool = actx.enter_context(tc.tile_pool(name="hp", bufs=2))
                apool = actx.enter_context(tc.tile_pool(name="ap", bufs=3))
                mpool = actx.enter_context(tc.tile_pool(name="mp", bufs=1))
                pap = actx.enter_context(
                    tc.tile_pool(name="pap", bufs=2, space="PSUM"))
                pal = actx.enter_context(
                    tc.tile_pool(name="pal", bufs=2, space="PSUM"))

                aw_sb = cpool.tile([128, 8, 2 * H], bf16)
                for kc in range(8):
                    nc.sync.dma_start(out=aw_sb[:, kc, :],
                                      in_=aw[kc * 128:(kc + 1) * 128, :])
                ab_sb = cpool.tile([128, 8], f32)
                nc.sync.dma_start(out=ab_sb, in_=ab_t)
                av_sb = cpool.tile([128, 8], bf16)
                nc.sync.dma_start(out=av_sb, in_=av_t)

                for tck in range(TOKCH):
                    l0 = tck * TOKL
                    hid_sb = hpool.tile([128, 8, TOKL, BS], bf16, tag="hsb")
                    for ch in range(8):
                        nc.sync.dma_start(out=hid_sb[:, ch, :, :],
                                          in_=hid[ch, :, l0:l0 + TOKL, :])
                    hflat = hid_sb.rearrange("p c l b -> p c (l b)")
                    ps_al = pal.tile([1, TOKL * BS], f32, tag="psal")
                    for m in range(8):
                        ps_a = pap.tile([128, TOKL * BS], f32, tag="psa")
                        for kc in range(8):
                            nc.tensor.matmul(
                                ps_a, aw_sb[:, kc, m * 128:(m + 1) * 128],
                                hflat[:, kc, :],
                                start=(kc == 0), stop=(kc == 7))
                        at_sb = apool.tile([128, TOKL * BS], bf16, tag="atsb")
                        nc.scalar.activation(at_sb, ps_a, AF.Tanh,
                                             bias=ab_sb[:, m:m + 1])
                        nc.tensor.matmul(ps_al, av_sb[:, m:m + 1], at_sb,
                                         start=(m == 0), stop=(m == 7))
                    al_sb = apool.tile([1, TOKL * BS], f32, tag="alsb")
                    nc.scalar.copy(al_sb, ps_al)
                    nc.sync.dma_start(
                        out=alpha_d[l0:l0 + TOKL, :],
                        in_=al_sb.rearrange("p (l b) -> p l b", l=TOKL))

                # softmax over l per b
                alv = mpool.tile([BS, L], f32)
                nc.sync.dma_start(out=alv, in_=alpha_d.rearrange("l b -> b l"))
                madd = mpool.tile([BS, L], f32)
                nc.sync.dma_start(out=madd, in_=maskadd)
                alm = mpool.tile([BS, L], f32)
                nc.vector.tensor_add(alm, alv, madd)
                negmax = mpool.tile([BS, 1], f32)
                nc.vector.tensor_reduce(negmax, alm, AX.X, ALU.max,
                                        negate=True)
                esb = mpool.tile([BS, L], f32)
                ssum = mpool.tile([BS, 1], f32)
                nc.scalar.activation(esb, alm, AF.Exp, bias=negmax,
                                     accum_out=ssum)
                rsum = mpool.tile([BS, 1], f32)
                nc.vector.reciprocal(rsum, ssum)
                attw = mpool.tile([BS, L], bf16)
                nc.vector.tensor_scalar_mul(attw, esb, rsum)
                nc.sync.dma_start(out=attw_d.rearrange("l b -> b l"), in_=attw)

                # sent = einsum over l
                import concourse.bass as bass
                attw_flat = attw_d.rearrange("l b -> (l b)")
                attw_bcast = bass.AP(tensor=attw_flat.tensor,
                                     offset=attw_flat.offset,
                                     ap=[[0, 128]] + list(attw_flat.ap))
                attw_rep = mpool.tile([128, L * BS], bf16)
                nc.sync.dma_start(out=attw_rep, in_=attw_bcast)
                arv = attw_rep.rearrange("p (l b) -> p l b", l=L)
                sent = mpool.tile([128, 8, BS], f32)
                for ch in range(8):
                    hfull = hpool.tile([128, L, BS], bf16, tag="hfull")
                    nc.sync.dma_start(out=hfull, in_=hid[ch, :, :, :])
                    mul_t = hpool.tile([128, L, BS], bf16, tag="mult")
                    nc.vector.tensor_mul(mul_t, hfull, arv)
                    nc.vector.tensor_reduce(
                        sent[:, ch, :], mul_t.rearrange("p l b -> p b l"),
                        AX.X, ALU.add)

                # classifier
                cw_sb = cpool.tile([128, 8, C], f32)
                for kc in range(8):
                    nc.sync.dma_start(out=cw_sb[:, kc, :],
                                      in_=cw[kc * 128:(kc + 1) * 128, :])
                cb_sb = cpool.tile([BS, C], f32)
                nc.sync.dma_start(out=cb_sb, in_=cb_rep)
                sent_c = mpool.tile([128, 8, BS], f32)
                nc.vector.tensor_copy(sent_c, sent)
                ps_c = pal.tile([BS, C], f32, tag="psc")
                for ch in range(8):
                    nc.tensor.matmul(ps_c, sent_c[:, ch, :], cw_sb[:, ch, :],
                                     start=(ch == 0), stop=(ch == 7))
                logits = mpool.tile([BS, C], f32)
                nc.vector.tensor_add(logits, ps_c, cb_sb)
                ngm = mpool.tile([BS, 1], f32)
                nc.vector.tensor_reduce(ngm, logits, AX.X, ALU.max,
                                        negate=True)
                e2 = mpool.tile([BS, C], f32)
                s2 = mpool.tile([BS, 1], f32)
                nc.scalar.activation(e2, logits, AF.Exp, bias=ngm,
                                     accum_out=s2)
                lns = mpool.tile([BS, 1], f32)
                nc.scalar.activation(lns, s2, AF.Ln)
                tmp1 = mpool.tile([BS, C], f32)
                nc.vector.tensor_scalar_add(tmp1, logits, ngm)
                res = mpool.tile([BS, C], f32)
                nc.vector.tensor_scalar_sub(res, tmp1, lns)
                nc.sync.dma_start(out=out, in_=res)
            if actx_loop is not None:
                actx_loop.__exit__(None, None, None)

    nc.compile()
    return nc


def _prep_host(x, mask, fWf, fbf, fWi, fbi, fWo, fbo, fWg, fbg,
               bWf, bbf, bWi, bbi, bWo, bbo, bWg, bbg,
               aW, ab, av, cW, cb):
    bf = ml_dtypes.bfloat16

    def aug(Ws):
        # [E+H, 4H]: rows 0..E-1 x-part, E..E+H-1 h-part
        m = np.zeros((E + H, 4 * H), np.float32)
        for g, Wg_ in enumerate(Ws):
            m[:, g * H:(g + 1) * H] = Wg_
        return m.astype(bf)

    def bias_block(bs):
        # [16, 128]: row k=(hc*4+gg) holds bias[gg*512 + hc*128 : +128]
        blk = np.zeros((16, 128), np.float32)
        for hc in range(4):
            for g in range(4):
                blk[hc * 4 + g] = np.asarray(bs[g], np.float32)[
                    hc * 128:(hc + 1) * 128]
        return blk

    wf_np = aug([fWf, fWi, fWo, fWg])
    wb_np = aug([bWf, bWi, bWo, bWg])
    bias_np = np.stack([bias_block([fbf, fbi, fbo, fbg]),
                        bias_block([bbf, bbi, bbo, bbg])]).astype(bf)
    # indicator: column (bank, hcq, gg, t, b) belongs to row k=(2*bank+hcq)*4+gg
    ind_np = np.zeros((16, 1024), np.float32)
    for bank in range(2):
        for hcq in range(2):
            for g in range(4):
                k = (2 * bank + hcq) * 4 + g
                c0 = bank * 512 + hcq * 256 + g * 64
                ind_np[k, c0:c0 + 64] = 1.0
    ind_np = ind_np.astype(bf)
    aw_np = np.asarray(aW, np.float32).astype(bf)
    ab_np = np.asarray(ab, np.float32).reshape(8, 128).T.copy()
    av_np = np.asarray(av, np.float32).reshape(8, 128).T.astype(bf).copy()
    cw_np = np.asarray(cW, np.float32).copy()
    cb_np = np.tile(np.asarray(cb, np.float32), (BS, 1))

    x = np.asarray(x, np.float32)
    mask = np.asarray(mask)
    in_maps = []
    for i in range(NCORES):
        sl = slice(i * BS, (i + 1) * BS)
        xs0 = x[sl].transpose(1, 2, 0).astype(bf)          # [L, E, BS]
        # window-major: [NW, E, W, BS]
        xs = np.ascontiguousarray(
            xs0.reshape(NW, W, E, BS).transpose(0, 2, 1, 3))
        xsr = np.ascontiguousarray(
            xs0[::-1].reshape(NW, W, E, BS).transpose(0, 2, 1, 3))
        ma = ((mask[sl].astype(np.float32) - 1.0) * 1e9)
        in_maps.append({
            "xT": xs, "xTr": xsr, "wf": wf_np, "wb": wb_np,
            "bias_blk": bias_np, "ind": ind_np,
            "aw": aw_np, "ab_t": ab_np, "av_t": av_np,
            "cw": cw_np, "cb_rep": cb_np, "maskadd": ma,
        })
    return in_maps


def kernel(**inputs):
    from concourse.bass_utils import run_bass_kernel_spmd
    if "nc" not in _cache:
        _cache["nc"] = _build_nc()
    nc = _cache["nc"]
    in_maps = _prep_host(**inputs)
    res = run_bass_kernel_spmd(nc, in_maps, core_ids=list(range(NCORES)))
    return np.concatenate([res.results[i]["out"] for i in range(NCORES)],
                          axis=0)

